# revision 1
# baseline (speedup 1.0000x reference)
# kernel.py — DecoderSourceTarget (gather-dot-sigmoid) on 8 Trainium2 NeuronCores.
#
# reference:
#   src = x[edge_label_index[0], :128]; dst = x[edge_label_index[1], 128:]
#   out = sigmoid(sum(src * dst, axis=-1))[:, None]          # [E, 1] f32
#
# Strategy (edge-parallel, x replicated):
#   - Host splits x into half-tables xs=x[:,:128], xd=x[:,128:], each stored
#     as 4 chunks of 25000 rows (so gather indices fit dma_gather's int16).
#   - The 1M edges are sharded contiguously across 8 cores (125k each); each
#     core's edges are bucket-sorted by (src_chunk, dst_chunk) on the host.
#     Bucket capacities (padded to 128, shared across cores) are baked into
#     the kernel; padding slots gather row 0 and are dropped on the host.
#   - Per bucket, edges are processed in sub-tiles of up to 2048: one
#     gpsimd.dma_gather pulls the 512B src rows from HBM into an SBUF tile
#     [128, q, 128] (slot j = (j%128, j//128)), same for dst; DVE multiplies
#     and reduces each 128-wide group to a logit; one sigmoid + one
#     contiguous DMA writes all logits at the end.
#   - Host maps slots back to edge order (pure numpy argsort bookkeeping).

import numpy as np

N_NODES = 100000
HALF = 128
N_EDGES = 1000000
N_CORES = 8
N_CHUNKS = 4
CHUNK = N_NODES // N_CHUNKS          # 25000 rows per table chunk
PER_CORE = N_EDGES // N_CORES        # 125000
SUB = 8192                           # max edges per gather call

_CACHE = {}


def _build_nc(caps):
    """caps: tuple of 16 bucket capacities (each a multiple of 128), shared
    by all cores. Returns a compiled Bacc module."""
    key = ("nc", caps)
    if key in _CACHE:
        return _CACHE[key]
    from contextlib import ExitStack

    from concourse import bacc, mybir, tile

    tot = sum(caps)
    cols = tot // 128

    nc = bacc.Bacc(
        "TRN2",
        target_bir_lowering=False,
        debug=False,
        num_devices=N_CORES,
    )
    xs_c = [
        nc.dram_tensor(f"xs{a}", [CHUNK, HALF], mybir.dt.float32,
                       kind="ExternalInput").ap()
        for a in range(N_CHUNKS)
    ]
    xd_c = [
        nc.dram_tensor(f"xd{b}", [CHUNK, HALF], mybir.dt.float32,
                       kind="ExternalInput").ap()
        for b in range(N_CHUNKS)
    ]
    isrc_d = nc.dram_tensor(
        "isrc", [128, tot // 16], mybir.dt.int16, kind="ExternalInput"
    ).ap()
    idst_d = nc.dram_tensor(
        "idst", [128, tot // 16], mybir.dt.int16, kind="ExternalInput"
    ).ap()
    out_d = nc.dram_tensor(
        "out", [128, cols], mybir.dt.float32, kind="ExternalOutput"
    ).ap()

    with tile.TileContext(nc) as tc, ExitStack() as ctx:
        fixed = ctx.enter_context(tc.tile_pool(name="fixed", bufs=1))
        work = ctx.enter_context(tc.tile_pool(name="work", bufs=2))

        isrc_sb = fixed.tile([128, tot // 16], mybir.dt.int16)
        idst_sb = fixed.tile([128, tot // 16], mybir.dt.int16)
        logits_sb = fixed.tile([128, cols], mybir.dt.float32)
        sig_sb = fixed.tile([128, cols], mybir.dt.float32)
        nc.sync.dma_start(isrc_sb[:], isrc_d)
        nc.sync.dma_start(idst_sb[:], idst_d)

        slot = 0
        for bucket, cap in enumerate(caps):
            a, b = divmod(bucket, N_CHUNKS)
            done = 0
            while done < cap:
                n = min(SUB, cap - done)
                q = n // 128
                scol = (slot + done) // 16       # idx column base
                lcol = (slot + done) // 128      # logit column base
                s_t = work.tile([128, q, HALF], mybir.dt.float32, tag="s")
                d_t = work.tile([128, q, HALF], mybir.dt.float32, tag="d")
                nc.gpsimd.dma_gather(
                    s_t[:], xs_c[a], isrc_sb[:, scol:scol + n // 16],
                    num_idxs=n, num_idxs_reg=n,
                    elem_size=HALF, elem_step=HALF, single_packet=False,
                )
                nc.gpsimd.dma_gather(
                    d_t[:], xd_c[b], idst_sb[:, scol:scol + n // 16],
                    num_idxs=n, num_idxs_reg=n,
                    elem_size=HALF, elem_step=HALF, single_packet=False,
                )
                # in-place product (same-element in-place is safe on DVE and
                # saves a third 4MB tile tag, letting 8192-edge tiles
                # double-buffer within SBUF)
                nc.vector.tensor_mul(s_t[:], s_t[:], d_t[:])
                nc.vector.tensor_reduce(
                    logits_sb[:, lcol:lcol + q],
                    s_t[:],
                    axis=mybir.AxisListType.X,
                    op=mybir.AluOpType.add,
                )
                done += n
            slot += cap

        nc.scalar.activation(
            sig_sb[:], logits_sb[:], mybir.ActivationFunctionType.Sigmoid
        )
        nc.sync.dma_start(out_d, sig_sb[:])

    nc.compile()
    _CACHE[key] = nc
    return nc


def _wrap_idx(idx_flat):
    """[tot] int16 -> [128, tot//16] int16 dma_gather layout: position j at
    [j%16, j//16], replicated to all 8 Q7 core groups."""
    w16 = idx_flat.reshape(-1, 16).T            # [16, tot/16]
    return np.ascontiguousarray(np.tile(w16, (8, 1)))


def _plan(src, dst):
    """Bucket-sort one core's edges by (src_chunk, dst_chunk).

    Returns (order, counts): `order` is the stable bucket sort permutation,
    `counts` the per-bucket edge counts."""
    key = (src // CHUNK) * N_CHUNKS + (dst // CHUNK)
    order = np.argsort(key, kind="stable")
    counts = np.bincount(key, minlength=N_CHUNKS * N_CHUNKS)
    return order, counts


def _make_run_data(x, edge_label_index):
    x = np.asarray(x, dtype=np.float32)
    eli = np.asarray(edge_label_index)
    assert x.shape == (N_NODES, 2 * HALF), x.shape
    assert eli.shape == (2, N_EDGES), eli.shape
    src = np.ascontiguousarray(eli[0]).astype(np.int64)
    dst = np.ascontiguousarray(eli[1]).astype(np.int64)
    assert src.min() >= 0 and src.max() < N_NODES
    assert dst.min() >= 0 and dst.max() < N_NODES

    chunks_s = [np.ascontiguousarray(x[a * CHUNK:(a + 1) * CHUNK, :HALF])
                for a in range(N_CHUNKS)]
    chunks_d = [np.ascontiguousarray(x[a * CHUNK:(a + 1) * CHUNK, HALF:])
                for a in range(N_CHUNKS)]

    plans = []
    all_counts = []
    for c in range(N_CORES):
        sl = slice(c * PER_CORE, (c + 1) * PER_CORE)
        order, counts = _plan(src[sl], dst[sl])
        plans.append(order)
        all_counts.append(counts)
    # shared capacities: max over cores, rounded up to 128 (min one tile row)
    counts_max = np.max(all_counts, axis=0)
    caps = tuple(int(-(-max(n, 1) // 128) * 128) for n in counts_max)
    tot = sum(caps)
    starts = np.concatenate([[0], np.cumsum(caps)[:-1]]).astype(np.int64)

    in_maps = []
    slot_maps = []
    for c in range(N_CORES):
        sl = slice(c * PER_CORE, (c + 1) * PER_CORE)
        s_c, d_c = src[sl], dst[sl]
        order, counts = plans[c], all_counts[c]
        key_sorted = ((s_c // CHUNK) * N_CHUNKS + (d_c // CHUNK))[order]
        group_start = np.concatenate([[0], np.cumsum(counts)[:-1]])
        ranks = np.arange(PER_CORE) - np.repeat(group_start, counts)
        slots = starts[key_sorted] + ranks          # slot of sorted edge i
        isrc_flat = np.zeros(tot, np.int16)
        idst_flat = np.zeros(tot, np.int16)
        isrc_flat[slots] = (s_c[order] % CHUNK).astype(np.int16)
        idst_flat[slots] = (d_c[order] % CHUNK).astype(np.int16)
        # edge -> slot map (undo the sort)
        edge_slot = np.empty(PER_CORE, np.int64)
        edge_slot[order] = slots
        slot_maps.append(edge_slot)
        im = {f"xs{a}": chunks_s[a] for a in range(N_CHUNKS)}
        im.update({f"xd{b}": chunks_d[b] for b in range(N_CHUNKS)})
        im["isrc"] = _wrap_idx(isrc_flat)
        im["idst"] = _wrap_idx(idst_flat)
        in_maps.append(im)
    return caps, in_maps, slot_maps


def _run(caps, in_maps, **kwargs):
    from concourse.bass_utils import run_bass_kernel_spmd

    nc = _build_nc(caps)
    return run_bass_kernel_spmd(nc, in_maps, core_ids=list(range(N_CORES)), **kwargs)


def kernel(x, edge_label_index):
    caps, in_maps, slot_maps = _make_run_data(x, edge_label_index)
    res = _run(caps, in_maps)
    parts = []
    for c in range(N_CORES):
        o = res.results[c]["out"]            # [128, cols]
        slot_vals = o.T.reshape(-1)          # slot j = o[j%128, j//128]
        parts.append(slot_vals[slot_maps[c]])
    return np.concatenate(parts).reshape(-1, 1).astype(np.float32)



# revision 2
# speedup vs baseline: 3.5330x; 3.5330x over previous
# kernel.py — DecoderSourceTarget (gather-dot-sigmoid) on 8 Trainium2 NeuronCores.
#
# reference:
#   src = x[edge_label_index[0], :128]; dst = x[edge_label_index[1], 128:]
#   out = sigmoid(sum(src * dst, axis=-1))[:, None]          # [E, 1] f32
#
# Strategy (edge-parallel, x replicated, multi-queue SWDGE gathers):
#   - Host splits x into bf16 half-tables xs=x[:,:128], xd=x[:,128:], each as
#     4 chunks of 25000 rows (so gather indices fit dma_gather's int16).
#   - The 1M edges are sharded contiguously across 8 cores (125k each); each
#     core's edges are bucket-sorted by (src_chunk, dst_chunk) on the host.
#     Bucket capacities (padded to 128, shared across cores) are baked into
#     the kernel; padding slots gather row 0 and are dropped on the host.
#   - Each bucket side (src rows, dst rows) is ONE gpsimd.dma_gather.  The
#     critical optimization: gathers are striped over the 4 SWDGE queues
#     (num_swdge_queues=4), whose descriptor generation runs on disjoint Q7
#     core pairs CONCURRENTLY (~4x the single-queue descriptor rate that
#     bounds the baseline).  A queue can only be reused after a
#     gpsimd.dma_reset() drain, so buckets are processed in rounds of two
#     (4 gathers on queues 0..3) with a drain between rounds.  Work tiles
#     alternate between two sets so DVE mul+reduce overlaps the next round.
#   - DVE multiplies src*dst (bf16) and reduces each 128-wide group to an
#     f32 logit; one sigmoid + one contiguous DMA writes all logits.
#   - Host maps slots back to edge order (pure numpy argsort bookkeeping).

import numpy as np

N_NODES = 100000
HALF = 128
N_EDGES = 1000000
N_CORES = 8
N_CHUNKS = 4
CHUNK = N_NODES // N_CHUNKS          # 25000 rows per table chunk
PER_CORE = N_EDGES // N_CORES        # 125000
N_BUCKETS = N_CHUNKS * N_CHUNKS

_CACHE = {}


def _build_nc(caps):
    """caps: tuple of 16 bucket capacities (each a multiple of 128), shared
    by all cores. Returns a compiled Bacc module."""
    key = ("nc", caps)
    if key in _CACHE:
        return _CACHE[key]
    from contextlib import ExitStack

    from concourse import bacc, mybir, tile

    tot = sum(caps)
    cols = tot // 128
    mcap = max(caps)

    nc = bacc.Bacc(
        "TRN2",
        target_bir_lowering=False,
        debug=False,
        num_devices=N_CORES,
        num_swdge_queues=4,
    )
    xs_c = [
        nc.dram_tensor(f"xs{a}", [CHUNK, HALF], mybir.dt.bfloat16,
                       kind="ExternalInput").ap()
        for a in range(N_CHUNKS)
    ]
    xd_c = [
        nc.dram_tensor(f"xd{b}", [CHUNK, HALF], mybir.dt.bfloat16,
                       kind="ExternalInput").ap()
        for b in range(N_CHUNKS)
    ]
    isrc_d = nc.dram_tensor(
        "isrc", [128, tot // 16], mybir.dt.int16, kind="ExternalInput"
    ).ap()
    idst_d = nc.dram_tensor(
        "idst", [128, tot // 16], mybir.dt.int16, kind="ExternalInput"
    ).ap()
    out_d = nc.dram_tensor(
        "out", [128, cols], mybir.dt.float32, kind="ExternalOutput"
    ).ap()

    with tile.TileContext(nc) as tc, ExitStack() as ctx:
        fixed = ctx.enter_context(tc.tile_pool(name="fixed", bufs=1))

        isrc_sb = fixed.tile([128, tot // 16], mybir.dt.int16)
        idst_sb = fixed.tile([128, tot // 16], mybir.dt.int16)
        logits_sb = fixed.tile([128, cols], mybir.dt.float32)
        sig_sb = fixed.tile([128, cols], mybir.dt.float32)
        nc.sync.dma_start(isrc_sb[:], isrc_d)
        nc.sync.dma_start(idst_sb[:], idst_d)

        # two alternating sets of (src,dst) tile pairs x 2 buckets per round
        wt = []
        for s in range(2):
            row = []
            for j in range(4):
                t = fixed.tile([128, mcap // 128, HALF], mybir.dt.bfloat16,
                               name=f"wt{s}_{j}")
                row.append(t)
            wt.append(row)

        starts = np.concatenate([[0], np.cumsum(caps)[:-1]]).astype(np.int64)
        rounds = [(i, i + 1) for i in range(0, N_BUCKETS, 2)]

        for r, bpair in enumerate(rounds):
            if r > 0:
                nc.gpsimd.dma_reset()
            tiles = wt[r % 2]
            # fire 4 gathers on queues 0..3
            for k, bucket in enumerate(bpair):
                a, b = divmod(bucket, N_CHUNKS)
                cap = caps[bucket]
                scol = starts[bucket] // 16
                s_t, d_t = tiles[2 * k], tiles[2 * k + 1]
                nc.gpsimd.dma_gather(
                    s_t[:, :cap // 128, :], xs_c[a],
                    isrc_sb[:, scol:scol + cap // 16],
                    num_idxs=cap, num_idxs_reg=cap,
                    elem_size=HALF, elem_step=HALF, single_packet=False,
                    queue_num=2 * k,
                )
                nc.gpsimd.dma_gather(
                    d_t[:, :cap // 128, :], xd_c[b],
                    idst_sb[:, scol:scol + cap // 16],
                    num_idxs=cap, num_idxs_reg=cap,
                    elem_size=HALF, elem_step=HALF, single_packet=False,
                    queue_num=2 * k + 1,
                )
            # consume: mul (in-place bf16) + reduce to f32 logits
            for k, bucket in enumerate(bpair):
                cap = caps[bucket]
                lcol = starts[bucket] // 128
                s_t, d_t = tiles[2 * k], tiles[2 * k + 1]
                nc.vector.tensor_mul(
                    s_t[:, :cap // 128, :], s_t[:, :cap // 128, :],
                    d_t[:, :cap // 128, :],
                )
                nc.vector.tensor_reduce(
                    logits_sb[:, lcol:lcol + cap // 128],
                    s_t[:, :cap // 128, :],
                    axis=mybir.AxisListType.X,
                    op=mybir.AluOpType.add,
                )

        nc.scalar.activation(
            sig_sb[:], logits_sb[:], mybir.ActivationFunctionType.Sigmoid
        )
        nc.sync.dma_start(out_d, sig_sb[:])

    nc.compile()
    _CACHE[key] = nc
    return nc


def _wrap_idx(idx_flat):
    """[tot] int16 -> [128, tot//16] int16 dma_gather layout: position j at
    [j%16, j//16], replicated to all 8 Q7 core groups."""
    w16 = idx_flat.reshape(-1, 16).T            # [16, tot/16]
    return np.ascontiguousarray(np.tile(w16, (8, 1)))


def _plan(src, dst):
    """Bucket-sort one core's edges by (src_chunk, dst_chunk)."""
    key = (src // CHUNK) * N_CHUNKS + (dst // CHUNK)
    order = np.argsort(key, kind="stable")
    counts = np.bincount(key, minlength=N_BUCKETS)
    return order, counts


def _make_run_data(x, edge_label_index):
    import ml_dtypes

    x = np.asarray(x, dtype=np.float32)
    eli = np.asarray(edge_label_index)
    assert x.shape == (N_NODES, 2 * HALF), x.shape
    assert eli.shape == (2, N_EDGES), eli.shape
    src = np.ascontiguousarray(eli[0]).astype(np.int64)
    dst = np.ascontiguousarray(eli[1]).astype(np.int64)
    assert src.min() >= 0 and src.max() < N_NODES
    assert dst.min() >= 0 and dst.max() < N_NODES

    xbf = x.astype(ml_dtypes.bfloat16)
    chunks_s = [np.ascontiguousarray(xbf[a * CHUNK:(a + 1) * CHUNK, :HALF])
                for a in range(N_CHUNKS)]
    chunks_d = [np.ascontiguousarray(xbf[a * CHUNK:(a + 1) * CHUNK, HALF:])
                for a in range(N_CHUNKS)]

    plans = []
    all_counts = []
    for c in range(N_CORES):
        sl = slice(c * PER_CORE, (c + 1) * PER_CORE)
        order, counts = _plan(src[sl], dst[sl])
        plans.append(order)
        all_counts.append(counts)
    # shared capacities: max over cores, rounded up to 128 (min one tile row)
    counts_max = np.max(all_counts, axis=0)
    caps = tuple(int(-(-max(n, 1) // 128) * 128) for n in counts_max)
    tot = sum(caps)
    starts = np.concatenate([[0], np.cumsum(caps)[:-1]]).astype(np.int64)

    in_maps = []
    slot_maps = []
    for c in range(N_CORES):
        sl = slice(c * PER_CORE, (c + 1) * PER_CORE)
        s_c, d_c = src[sl], dst[sl]
        order, counts = plans[c], all_counts[c]
        key_sorted = ((s_c // CHUNK) * N_CHUNKS + (d_c // CHUNK))[order]
        group_start = np.concatenate([[0], np.cumsum(counts)[:-1]])
        ranks = np.arange(PER_CORE) - np.repeat(group_start, counts)
        slots = starts[key_sorted] + ranks          # slot of sorted edge i
        isrc_flat = np.zeros(tot, np.int16)
        idst_flat = np.zeros(tot, np.int16)
        isrc_flat[slots] = (s_c[order] % CHUNK).astype(np.int16)
        idst_flat[slots] = (d_c[order] % CHUNK).astype(np.int16)
        # edge -> slot map (undo the sort)
        edge_slot = np.empty(PER_CORE, np.int64)
        edge_slot[order] = slots
        slot_maps.append(edge_slot)
        im = {f"xs{a}": chunks_s[a] for a in range(N_CHUNKS)}
        im.update({f"xd{b}": chunks_d[b] for b in range(N_CHUNKS)})
        im["isrc"] = _wrap_idx(isrc_flat)
        im["idst"] = _wrap_idx(idst_flat)
        in_maps.append(im)
    return caps, in_maps, slot_maps


def _run(caps, in_maps, **kwargs):
    from concourse.bass_utils import run_bass_kernel_spmd

    nc = _build_nc(caps)
    return run_bass_kernel_spmd(nc, in_maps, core_ids=list(range(N_CORES)), **kwargs)


def kernel(x, edge_label_index):
    caps, in_maps, slot_maps = _make_run_data(x, edge_label_index)
    res = _run(caps, in_maps)
    parts = []
    for c in range(N_CORES):
        o = res.results[c]["out"]            # [128, cols]
        slot_vals = o.T.reshape(-1)          # slot j = o[j%128, j//128]
        parts.append(slot_vals[slot_maps[c]])
    return np.concatenate(parts).reshape(-1, 1).astype(np.float32)


# revision 4
# speedup vs baseline: 3.5892x; 1.0159x over previous
# kernel.py — DecoderSourceTarget (gather-dot-sigmoid) on 8 Trainium2 NeuronCores.
#
# reference:
#   src = x[edge_label_index[0], :128]; dst = x[edge_label_index[1], 128:]
#   out = sigmoid(sum(src * dst, axis=-1))[:, None]          # [E, 1] f32
#
# Strategy (edge-parallel, x replicated, multi-queue SWDGE gathers):
#   - Host splits x into bf16 half-tables xs=x[:,:128], xd=x[:,128:], each as
#     4 chunks of 25000 rows (so gather indices fit dma_gather's int16).
#   - The 1M edges are sharded contiguously across 8 cores (125k each); each
#     core's edges are bucket-sorted by (src_chunk, dst_chunk) on the host.
#     Bucket capacities (padded to 128, shared across cores) are baked into
#     the kernel; padding slots gather row 0 and are dropped on the host.
#   - Each bucket side (src rows, dst rows) is ONE gpsimd.dma_gather.  The
#     critical optimization: gathers are striped over the 4 SWDGE queues
#     (num_swdge_queues=4), whose descriptor generation runs on disjoint Q7
#     core pairs CONCURRENTLY (~4x the single-queue descriptor rate that
#     bounds the baseline).  A queue can only be reused after a
#     gpsimd.dma_reset() drain, so buckets are processed in rounds of two
#     (4 gathers on queues 0..3) with a drain between rounds.  Work tiles
#     alternate between two sets so DVE mul+reduce overlaps the next round.
#   - DVE multiplies src*dst (bf16) and reduces each 128-wide group to an
#     f32 logit; one sigmoid + one contiguous DMA writes all logits.
#   - Host maps slots back to edge order (pure numpy argsort bookkeeping).

import numpy as np

N_NODES = 100000
HALF = 128
N_EDGES = 1000000
N_CORES = 8
N_CHUNKS = 4
CHUNK = N_NODES // N_CHUNKS          # 25000 rows per table chunk
PER_CORE = N_EDGES // N_CORES        # 125000
N_BUCKETS = N_CHUNKS * N_CHUNKS

_CACHE = {}


def _build_nc(caps):
    """caps: tuple of 16 bucket capacities (each a multiple of 128), shared
    by all cores. Returns a compiled Bacc module."""
    key = ("nc", caps)
    if key in _CACHE:
        return _CACHE[key]
    from contextlib import ExitStack

    from concourse import bacc, mybir, tile

    tot = sum(caps)
    cols = tot // 128
    mcap = max(caps)

    nc = bacc.Bacc(
        "TRN2",
        target_bir_lowering=False,
        debug=False,
        num_devices=N_CORES,
        num_swdge_queues=4,
    )
    xs_c = [
        nc.dram_tensor(f"xs{a}", [CHUNK, HALF], mybir.dt.bfloat16,
                       kind="ExternalInput").ap()
        for a in range(N_CHUNKS)
    ]
    xd_c = [
        nc.dram_tensor(f"xd{b}", [CHUNK, HALF], mybir.dt.bfloat16,
                       kind="ExternalInput").ap()
        for b in range(N_CHUNKS)
    ]
    isrc_d = nc.dram_tensor(
        "isrc", [128, tot // 16], mybir.dt.int16, kind="ExternalInput"
    ).ap()
    idst_d = nc.dram_tensor(
        "idst", [128, tot // 16], mybir.dt.int16, kind="ExternalInput"
    ).ap()
    out_d = nc.dram_tensor(
        "out", [128, cols], mybir.dt.float32, kind="ExternalOutput"
    ).ap()

    with tile.TileContext(nc) as tc, ExitStack() as ctx:
        fixed = ctx.enter_context(tc.tile_pool(name="fixed", bufs=1))

        # split the index load so round 1 (buckets 0,1) can start before the
        # bulk of the index table arrives
        head = (caps[0] + caps[1]) // 16
        isrc_sb = fixed.tile([128, head], mybir.dt.int16)
        idst_sb = fixed.tile([128, head], mybir.dt.int16)
        isrc2_sb = fixed.tile([128, tot // 16 - head], mybir.dt.int16)
        idst2_sb = fixed.tile([128, tot // 16 - head], mybir.dt.int16)
        logits_sb = fixed.tile([128, cols], mybir.dt.float32)
        sig_sb = fixed.tile([128, cols], mybir.dt.float32)
        nc.sync.dma_start(isrc_sb[:], isrc_d[:, :head])
        nc.sync.dma_start(idst_sb[:], idst_d[:, :head])
        nc.sync.dma_start(isrc2_sb[:], isrc_d[:, head:])
        nc.sync.dma_start(idst2_sb[:], idst_d[:, head:])

        def idx_slice(side, scol, n):
            """int16 idx tile slice for absolute column range [scol, scol+n)."""
            t1, t2 = (isrc_sb, isrc2_sb) if side == 0 else (idst_sb, idst2_sb)
            if scol + n <= head:
                return t1[:, scol:scol + n]
            assert scol >= head, (scol, n, head)
            return t2[:, scol - head:scol - head + n]

        # two alternating sets of (src,dst) tile pairs x 2 buckets per round
        wt = []
        for s in range(2):
            row = []
            for j in range(4):
                t = fixed.tile([128, mcap // 128, HALF], mybir.dt.bfloat16,
                               name=f"wt{s}_{j}")
                row.append(t)
            wt.append(row)

        starts = np.concatenate([[0], np.cumsum(caps)[:-1]]).astype(np.int64)

        # rounds of two buckets (4 gathers, queues 0..3).  The final bucket
        # pair is split 3/4 + 1/4 into two sub-rounds (same tile set, same
        # queues) so the tail — the last async generation the final DVE and
        # sigmoid must wait for — is short.
        sched = []  # round = [(bucket, off, n), (bucket, off, n)]
        for i in range(0, N_BUCKETS - 2, 2):
            sched.append([(i, 0, caps[i]), (i + 1, 0, caps[i + 1])])
        b14, b15 = N_BUCKETS - 2, N_BUCKETS - 1
        c14 = max(128, caps[b14] * 3 // 4 // 128 * 128)
        c15 = max(128, caps[b15] * 3 // 4 // 128 * 128)
        sched.append([(b14, 0, c14), (b15, 0, c15)])
        sched.append([(b14, c14, caps[b14] - c14), (b15, c15, caps[b15] - c15)])
        n_norm = len(sched) - 1  # rounds n_norm-1 and n_norm share a tile set

        def consume(tiles, k, bucket):
            cap = caps[bucket]
            lcol = starts[bucket] // 128
            s_t, d_t = tiles[2 * k], tiles[2 * k + 1]
            nc.vector.tensor_mul(
                s_t[:, :cap // 128, :], s_t[:, :cap // 128, :],
                d_t[:, :cap // 128, :],
            )
            nc.vector.tensor_reduce(
                logits_sb[:, lcol:lcol + cap // 128],
                s_t[:, :cap // 128, :],
                axis=mybir.AxisListType.X,
                op=mybir.AluOpType.add,
            )

        for r, round_ in enumerate(sched):
            if r > 0:
                nc.gpsimd.dma_reset()
            tiles = wt[min(r, n_norm - 1) % 2]
            for k, (bucket, off, n) in enumerate(round_):
                if n == 0:
                    continue
                a, b = divmod(bucket, N_CHUNKS)
                scol = (starts[bucket] + off) // 16
                toff = off // 128
                s_t, d_t = tiles[2 * k], tiles[2 * k + 1]
                nc.gpsimd.dma_gather(
                    s_t[:, toff:toff + n // 128, :], xs_c[a],
                    idx_slice(0, scol, n // 16),
                    num_idxs=n, num_idxs_reg=n,
                    elem_size=HALF, elem_step=HALF, single_packet=False,
                    queue_num=2 * k,
                )
                nc.gpsimd.dma_gather(
                    d_t[:, toff:toff + n // 128, :], xd_c[b],
                    idx_slice(1, scol, n // 16),
                    num_idxs=n, num_idxs_reg=n,
                    elem_size=HALF, elem_step=HALF, single_packet=False,
                    queue_num=2 * k + 1,
                )
            if r < n_norm - 1:
                # both buckets of this round fully gathered: consume now
                for k, (bucket, _, _) in enumerate(round_):
                    consume(tiles, k, bucket)
            elif r == n_norm:
                # split pair complete after the second sub-round
                for k, bucket in enumerate((b14, b15)):
                    consume(tiles, k, bucket)

        nc.scalar.activation(
            sig_sb[:], logits_sb[:], mybir.ActivationFunctionType.Sigmoid
        )
        nc.sync.dma_start(out_d, sig_sb[:])

    nc.compile()
    _CACHE[key] = nc
    return nc


def _wrap_idx(idx_flat):
    """[tot] int16 -> [128, tot//16] int16 dma_gather layout: position j at
    [j%16, j//16], replicated to all 8 Q7 core groups."""
    w16 = idx_flat.reshape(-1, 16).T            # [16, tot/16]
    return np.ascontiguousarray(np.tile(w16, (8, 1)))


def _plan(src, dst):
    """Bucket-sort one core's edges by (src_chunk, dst_chunk)."""
    key = (src // CHUNK) * N_CHUNKS + (dst // CHUNK)
    order = np.argsort(key, kind="stable")
    counts = np.bincount(key, minlength=N_BUCKETS)
    return order, counts


def _make_run_data(x, edge_label_index):
    import ml_dtypes

    x = np.asarray(x, dtype=np.float32)
    eli = np.asarray(edge_label_index)
    assert x.shape == (N_NODES, 2 * HALF), x.shape
    assert eli.shape == (2, N_EDGES), eli.shape
    src = np.ascontiguousarray(eli[0]).astype(np.int64)
    dst = np.ascontiguousarray(eli[1]).astype(np.int64)
    assert src.min() >= 0 and src.max() < N_NODES
    assert dst.min() >= 0 and dst.max() < N_NODES

    xbf = x.astype(ml_dtypes.bfloat16)
    chunks_s = [np.ascontiguousarray(xbf[a * CHUNK:(a + 1) * CHUNK, :HALF])
                for a in range(N_CHUNKS)]
    chunks_d = [np.ascontiguousarray(xbf[a * CHUNK:(a + 1) * CHUNK, HALF:])
                for a in range(N_CHUNKS)]

    plans = []
    all_counts = []
    for c in range(N_CORES):
        sl = slice(c * PER_CORE, (c + 1) * PER_CORE)
        order, counts = _plan(src[sl], dst[sl])
        plans.append(order)
        all_counts.append(counts)
    # shared capacities: max over cores, rounded up to 128 (min one tile row)
    counts_max = np.max(all_counts, axis=0)
    caps = tuple(int(-(-max(n, 1) // 128) * 128) for n in counts_max)
    tot = sum(caps)
    starts = np.concatenate([[0], np.cumsum(caps)[:-1]]).astype(np.int64)

    in_maps = []
    slot_maps = []
    for c in range(N_CORES):
        sl = slice(c * PER_CORE, (c + 1) * PER_CORE)
        s_c, d_c = src[sl], dst[sl]
        order, counts = plans[c], all_counts[c]
        key_sorted = ((s_c // CHUNK) * N_CHUNKS + (d_c // CHUNK))[order]
        group_start = np.concatenate([[0], np.cumsum(counts)[:-1]])
        ranks = np.arange(PER_CORE) - np.repeat(group_start, counts)
        slots = starts[key_sorted] + ranks          # slot of sorted edge i
        isrc_flat = np.zeros(tot, np.int16)
        idst_flat = np.zeros(tot, np.int16)
        isrc_flat[slots] = (s_c[order] % CHUNK).astype(np.int16)
        idst_flat[slots] = (d_c[order] % CHUNK).astype(np.int16)
        # edge -> slot map (undo the sort)
        edge_slot = np.empty(PER_CORE, np.int64)
        edge_slot[order] = slots
        slot_maps.append(edge_slot)
        im = {f"xs{a}": chunks_s[a] for a in range(N_CHUNKS)}
        im.update({f"xd{b}": chunks_d[b] for b in range(N_CHUNKS)})
        im["isrc"] = _wrap_idx(isrc_flat)
        im["idst"] = _wrap_idx(idst_flat)
        in_maps.append(im)
    return caps, in_maps, slot_maps


def _run(caps, in_maps, **kwargs):
    from concourse.bass_utils import run_bass_kernel_spmd

    nc = _build_nc(caps)
    return run_bass_kernel_spmd(nc, in_maps, core_ids=list(range(N_CORES)), **kwargs)


def kernel(x, edge_label_index):
    caps, in_maps, slot_maps = _make_run_data(x, edge_label_index)
    res = _run(caps, in_maps)
    parts = []
    for c in range(N_CORES):
        o = res.results[c]["out"]            # [128, cols]
        slot_vals = o.T.reshape(-1)          # slot j = o[j%128, j//128]
        parts.append(slot_vals[slot_maps[c]])
    return np.concatenate(parts).reshape(-1, 1).astype(np.float32)
